# revision 1
# baseline (speedup 1.0000x reference)
"""Transformer block kernel for TRN2 (Bass/Tile), one batch element per core.

Computes (per core, x [1024, 768] f32):
    h  = LN(x) (gamma/beta pre-folded into weights on host)
    qk = h @ qkw + qkb ; q = qk[:, :768], k = qk[:, 768:]  (head-major 12x64)
    v  = h @ vw                 (v bias folded into proj bias on host)
    S^T[m,n] = (k_m . q_n) / 8 ;  P = exp(S^T)   (no max subtraction; scores are small)
    oe = [v; 1]^T @ P  -> rows 0..63 = unnormalized o^T, row 64 = softmax denom
    o^T = oe[0:64] / denom
    x1 = x + o @ pw + pb
    h2 = LN2(x1) (folded)
    out = x1 + gelu(h2 @ f1w + f1b) @ f2w + f2b

Layout convention: "feature-major" tensors are [feat_on_partitions, tokens] SBUF
tiles; token-major are [tokens_on_partitions, feat]. LN / residual are
token-major; matmuls contract over partitions so projections run feature-major.
"""

import sys
from contextlib import ExitStack

if "/opt/trn_rl_repo" not in sys.path:
    sys.path.insert(0, "/opt/trn_rl_repo")

import numpy as np

import concourse.bass as bass
import concourse.mybir as mybir
from concourse.masks import make_identity

F32 = mybir.dt.float32
F32R = mybir.dt.float32r
BF16 = mybir.dt.bfloat16
AF = mybir.ActivationFunctionType
ALU = mybir.AluOpType

P = 128
EMB = 768
SEQ = 1024
NH = 12
HD = 64
MLPD = 3072
EC = EMB // P      # 6 embedding chunks
NT = SEQ // P      # 8 token tiles
NC2 = SEQ // 512   # 2 token n-chunks
HC = MLPD // P     # 24 hidden chunks
HP = NH // 2       # 6 head pairs
EPS = 1e-5
SCALE = HD ** -0.5


def r32(ap):
    """Identity; matmul operands are declared float32r at allocation."""
    return ap


def _ln_stats(nc, x_ap, mv, stats, eps_t):
    """bn stats + rstd for one [128, EMB] tile; mv = [mean, rstd]."""
    xg = x_ap.rearrange("p (g d) -> p g d", d=256)
    for g in range(3):
        nc.vector.bn_stats(out=stats[:, g, :], in_=xg[:, g, :])
    nc.vector.bn_aggr(out=mv, in_=stats)
    # rstd = 1/sqrt(var + eps); Sqrt on ACT (one table set), exact recip on DVE
    # ([128,1] is one element per lane - fast)
    nc.scalar.activation(out=mv[:, 1:2], in_=mv[:, 1:2], func=AF.Sqrt, bias=eps_t, scale=1.0)
    nc.vector.reciprocal(out=mv[:, 1:2], in_=mv[:, 1:2])


def _ln_apply(nc, x_ap, h_out, mv):
    nc.vector.tensor_scalar(
        out=h_out,
        in0=x_ap,
        scalar1=mv[:, 0:1],
        scalar2=mv[:, 1:2],
        op0=ALU.subtract,
        op1=ALU.mult,
    )


def _transpose_to_featmajor(nc, tc, pool_ps, pool_sb, src_tok, dstT, t):
    """PE-transpose token-major src_tok [128, EMB] into dstT [:, e, t*128:(t+1)*128]."""
    ident = tc._block_ident
    for group_start, group_n in ((0, 4), (4, 2)):
        ptr = pool_ps.tile([P, 4 * P], BF16, tag="tr", name=f"ptr_t{t}_{group_start}")
        for j in range(group_n):
            e = group_start + j
            nc.tensor.transpose(
                ptr[:, j * P:(j + 1) * P],
                src_tok[:, e * P:(e + 1) * P],
                ident,
            )
        nc.scalar.copy(
            out=dstT[:, group_start:group_start + group_n, t * P:(t + 1) * P],
            in_=ptr[:, :group_n * P].rearrange("p (j q) -> p j q", q=P),
        )


def build_block(tc, outs, ins):
    nc = tc.nc
    x_d = ins["x"]
    qkw_d, qkb_d = ins["qkw"], ins["qkb"]
    vw_d = ins["vw"]
    pw_d, pb_d = ins["pw"], ins["pb"]
    f1w_d, f1b_d = ins["f1w"], ins["f1b"]
    f2w_d, f2b_d = ins["f2w"], ins["f2b"]
    out_d = outs["out"]

    with ExitStack() as ctx:
        consts = ctx.enter_context(tc.tile_pool(name="consts", bufs=1))
        ident = consts.tile([P, P], BF16)
        make_identity(nc, ident)
        tc._block_ident = ident
        eps_t = consts.tile([P, 1], F32)
        nc.vector.memset(eps_t, EPS)
        qkb_sb = consts.tile([P, 2 * EC], F32)
        pb_sb = consts.tile([P, EC], F32)
        f1b_sb = consts.tile([P, HC], F32)
        f2b_sb = consts.tile([P, EC], F32)

        # Persistent SBUF tensors
        glob = ctx.enter_context(tc.tile_pool(name="glob", bufs=1))
        x1 = glob.tile([P, NT, EMB], F32)            # residual stream (starts as x)
        actT = glob.tile([P, EC, SEQ], BF16, tag="actT")  # hT, later h2T reuses slot

        attn_glob = ctx.enter_context(tc.tile_pool(name="attn_glob", bufs=1))
        vext = attn_glob.tile([P, NT, NH, HD + 1], BF16)
        oT = attn_glob.tile([P, EC, SEQ], BF16)      # attention out, feature-major
        vw_sb = attn_glob.tile([P, EC, EMB], BF16)
        pw_sb = attn_glob.tile([P, EC, EMB], BF16)

        work = ctx.enter_context(tc.tile_pool(name="work", bufs=3))
        stat_pool = ctx.enter_context(tc.tile_pool(name="stat", bufs=4))

        # ---- load x into x1 (x1 is BOTH the LN1 input and the residual acc) ----
        x_r = x_d.rearrange("(t p) e -> p t e", p=P)
        for t in range(NT):
            nc.sync.dma_start(out=x1[:, t, :], in_=x_r[:, t, :])

        # ================= Phase A: LN1 + transpose to hT =================
        with tc.tile_pool(name="psA", space="PSUM", bufs=2) as psA:
            hs, mvs = [], []
            for t in range(NT):
                mv = stat_pool.tile([P, 2], F32, tag="mv", bufs=NT, name=f"mv1_{t}")
                stats = stat_pool.tile([P, 3, 6], F32, tag="stats", name=f"st1_{t}")
                _ln_stats(nc, x1[:, t, :], mv, stats, eps_t)
                mvs.append(mv)
            for t in range(NT):
                h_t = work.tile([P, EMB], BF16, tag="h", bufs=NT, name=f"h_{t}")
                _ln_apply(nc, x1[:, t, :], h_t, mvs[t])
                hs.append(h_t)
            for t in range(NT):
                _transpose_to_featmajor(nc, tc, psA, work, hs[t], actT, t)

        # weights / biases (emitted after x+LN so the x DMAs win the queues)
        nc.sync.dma_start(out=vw_sb, in_=vw_d.rearrange("(kc p) o -> p kc o", p=P))
        nc.sync.dma_start(out=qkb_sb, in_=qkb_d.rearrange("(m p) -> p m", p=P))
        nc.sync.dma_start(out=pb_sb, in_=pb_d.rearrange("(m p) -> p m", p=P))
        nc.sync.dma_start(out=f1b_sb, in_=f1b_d.rearrange("(m p) -> p m", p=P))
        nc.sync.dma_start(out=f2b_sb, in_=f2b_d.rearrange("(m p) -> p m", p=P))

        # ================= Phase B: v projection + attention =================
        with tc.tile_pool(name="psB", space="PSUM", bufs=2) as psB:
            # ---- v = h @ vw (token-major), packed into vext with ones column ----
            nc.vector.memset(vext[:, :, :, HD:HD + 1], 1.0)
            for t in range(NT):
                pv = psB.tile([P, 2, 512], F32, tag="mm2", name=f"pv_{t}")
                for half, (c0, cw) in enumerate(((0, 512), (512, 256))):
                    for e in range(EC):
                        nc.tensor.matmul(
                            pv[:, half, :cw],
                            actT[:, e, t * P:(t + 1) * P],
                            vw_sb[:, e, c0:c0 + cw],
                            start=(e == 0),
                            stop=(e == EC - 1),
                        )
                nc.vector.tensor_copy(
                    out=vext[:, t, 0:8, 0:HD],
                    in_=pv[:, 0, :].rearrange("p (h d) -> p h d", d=HD),
                )
                nc.vector.tensor_copy(
                    out=vext[:, t, 8:12, 0:HD],
                    in_=pv[:, 1, 0:256].rearrange("p (h d) -> p h d", d=HD),
                )

            nc.sync.dma_start(out=pw_sb, in_=pw_d.rearrange("(kc p) e -> p kc e", p=P))

            # ---- per head-pair: qk projection (prefetched one pair ahead),
            # ---- then attention for 2 heads
            qkw_r = qkw_d.rearrange("(kc p) o -> p kc o", p=P)

            def emit_qk(hp):
                qkT = {}
                for role, m in (("q", hp), ("k", HP + hp)):
                    wch = work.tile([P, EC, P], BF16, tag="wchunk", name=f"qkw_{role}{hp}")
                    nc.sync.dma_start(out=wch, in_=qkw_r[:, :, m * P:(m + 1) * P])
                    dst = work.tile([P, SEQ], BF16, tag="qkT", bufs=4, name=f"{role}T_{hp}")
                    for n in range(NC2):
                        pqk = psB.tile([P, 512], F32, tag="pqk", bufs=2, name=f"pqk_{role}{hp}n{n}")
                        for e in range(EC):
                            nc.tensor.matmul(
                                pqk,
                                wch[:, e, :],
                                actT[:, e, n * 512:(n + 1) * 512],
                                start=(e == 0),
                                stop=(e == EC - 1),
                            )
                        nc.vector.tensor_scalar_add(
                            out=dst[:, n * 512:(n + 1) * 512],
                            in0=pqk,
                            scalar1=qkb_sb[:, m:m + 1],
                        )
                    qkT[role] = dst
                return qkT

            qkT = emit_qk(0)
            for hp in range(HP):
                cur = qkT
                if hp + 1 < HP:
                    qkT = emit_qk(hp + 1)

                ous = {}
                dpack = stat_pool.tile([4, 512], F32, tag="dpack", bufs=2, name=f"dp_{hp}")
                for sub in range(2):
                    h = 2 * hp + sub
                    doff = sub * HD
                    qs = cur["q"][doff:doff + HD, :]
                    ks = cur["k"][doff:doff + HD, :]
                    po = [
                        psB.tile([P, 512], F32, tag="oacc", bufs=2, name=f"po_h{h}n{n}")
                        for n in range(NC2)
                    ]
                    for mt in range(NT):
                        ps = psB.tile([P, 2, 512], F32, tag="mm2", name=f"ps_h{h}m{mt}")
                        for n in range(NC2):
                            nc.tensor.matmul(
                                ps[:, n, :],
                                ks[:, mt * P:(mt + 1) * P],
                                qs[:, n * 512:(n + 1) * 512],
                                start=True,
                                stop=True,
                            )
                        pp = work.tile([P, 2, 512], BF16, tag="ppair", bufs=4, name=f"pp_h{h}m{mt}")
                        nc.scalar.activation(out=pp, in_=ps, func=AF.Exp, scale=SCALE)
                        for n in range(NC2):
                            nc.tensor.matmul(
                                po[n][0:HD + 1, :],
                                vext[:, mt, h, :],
                                pp[:, n, :],
                                start=(mt == 0),
                                stop=(mt == NT - 1),
                            )
                    for n in range(NC2):
                        # copy out of PSUM right away so the accumulator bank
                        # recycles without waiting on the normalize chain
                        ou = work.tile([HD + 1, 512], F32, tag="ou", bufs=8, name=f"ou_h{h}n{n}")
                        nc.vector.tensor_copy(out=ou, in_=po[n][0:HD + 1, :])
                        idx = sub * NC2 + n
                        nc.sync.dma_start(out=dpack[idx:idx + 1, :], in_=ou[HD:HD + 1, :])
                        ous[idx] = ou
                # one exact reciprocal for the whole head-pair's denominators
                rpack = stat_pool.tile([4, 512], F32, tag="rpack", bufs=2, name=f"rp_{hp}")
                nc.vector.reciprocal(out=rpack, in_=dpack)
                for sub in range(2):
                    doff = sub * HD
                    for n in range(NC2):
                        idx = sub * NC2 + n
                        rtmp = stat_pool.tile([1, 512], F32, tag="rtmp", bufs=4, name=f"rt_{hp}i{idx}")
                        nc.sync.dma_start(out=rtmp, in_=rpack[idx:idx + 1, :])
                        rb = work.tile([HD, 512], F32, tag="rb", bufs=4, name=f"rb_{hp}i{idx}")
                        nc.gpsimd.partition_broadcast(rb, rtmp)
                        nc.vector.tensor_tensor(
                            out=oT[doff:doff + HD, hp, n * 512:(n + 1) * 512],
                            in0=ous[idx][0:HD, :],
                            in1=rb,
                            op=ALU.mult,
                        )

        # ====== Phase C: proj + residual + LN2, one 512-token chunk at a time ======
        with tc.tile_pool(name="psC", space="PSUM", bufs=2) as psC:
            for n in range(NC2):
                for me in range(EC):
                    ppr = psC.tile([P, 512], F32, tag="mm", name=f"ppr_{me}_{n}")
                    for kc in range(EC):
                        nc.tensor.matmul(
                            ppr,
                            pw_sb[:, kc, me * P:(me + 1) * P],
                            oT[:, kc, n * 512:(n + 1) * 512],
                            start=(kc == 0),
                            stop=(kc == EC - 1),
                        )
                    prn = work.tile([P, 512], BF16, tag="prn", name=f"prn_{me}_{n}")
                    nc.scalar.activation(
                        out=prn, in_=ppr, func=AF.Identity, bias=pb_sb[:, me:me + 1]
                    )
                    ptr = psC.tile([P, 4, P], BF16, tag="tr", name=f"trp_{me}_{n}")
                    for j in range(4):
                        nc.tensor.transpose(ptr[:, j, :], prn[:, j * P:(j + 1) * P], ident)
                    nc.vector.tensor_tensor(
                        out=x1[:, 4 * n:4 * n + 4, me * P:(me + 1) * P],
                        in0=x1[:, 4 * n:4 * n + 4, me * P:(me + 1) * P],
                        in1=ptr,
                        op=ALU.add,
                    )
                hs2, mvs2 = [], []
                for j in range(4):
                    t = 4 * n + j
                    mv = stat_pool.tile([P, 2], F32, tag="mv", bufs=NT, name=f"mv2_{t}")
                    stats = stat_pool.tile([P, 3, 6], F32, tag="stats", name=f"st2_{t}")
                    _ln_stats(nc, x1[:, t, :], mv, stats, eps_t)
                    mvs2.append(mv)
                for j in range(4):
                    t = 4 * n + j
                    h_t = work.tile([P, EMB], BF16, tag="h", bufs=NT, name=f"h2_{t}")
                    _ln_apply(nc, x1[:, t, :], h_t, mvs2[j])
                    hs2.append(h_t)
                for j in range(4):
                    _transpose_to_featmajor(nc, tc, psC, work, hs2[j], actT, 4 * n + j)
        h2T = actT

        # ================= Phase F: MLP + residual + output =================
        out_r = out_d.rearrange("(t p) e -> p t e", p=P)
        f1w_r = f1w_d.rearrange("(kc p) o -> p kc o", p=P)
        f2w_r = f2w_d.rearrange("(hc p) e -> p hc e", p=P)
        with tc.tile_pool(name="psF", space="PSUM", bufs=1) as psF:
            for n in range(NC2):
                acc = [
                    psF.tile([P, 2, 512], F32, tag=f"acc{i}", bufs=1, name=f"acc_{n}_{i}")
                    for i in range(3)
                ]

                def acc_sl(e):
                    return acc[e // 2][:, e % 2, :]

                for hc in range(HC):
                    w1 = work.tile([P, EC, P], BF16, tag="wchunk", name=f"f1w_{n}_{hc}")
                    nc.sync.dma_start(out=w1, in_=f1w_r[:, :, hc * P:(hc + 1) * P])
                    w2 = work.tile([P, EMB], BF16, tag="w2chunk", name=f"f2w_{n}_{hc}")
                    nc.sync.dma_start(out=w2, in_=f2w_r[:, hc, :])
                    pf1 = psF.tile([P, 512], F32, tag="f1", bufs=2, name=f"pf1_{n}_{hc}")
                    for e in range(EC):
                        nc.tensor.matmul(
                            pf1,
                            w1[:, e, :],
                            h2T[:, e, n * 512:(n + 1) * 512],
                            start=(e == 0),
                            stop=(e == EC - 1),
                        )
                    a = work.tile([P, 512], BF16, tag="act", name=f"act_{n}_{hc}")
                    nc.scalar.activation(
                        out=a, in_=pf1, func=AF.Gelu, bias=f1b_sb[:, hc:hc + 1]
                    )
                    for e in range(EC):
                        nc.tensor.matmul(
                            acc_sl(e),
                            w2[:, e * P:(e + 1) * P],
                            a,
                            start=(hc == 0),
                            stop=(hc == HC - 1),
                        )
                for e in range(EC):
                    fr = work.tile([P, 512], BF16, tag="prn", name=f"fr_{n}_{e}")
                    nc.scalar.activation(
                        out=fr, in_=acc_sl(e), func=AF.Identity, bias=f2b_sb[:, e:e + 1]
                    )
                    ptr = psF.tile([P, 4, P], BF16, tag="f1", bufs=2, name=f"trf_{n}_{e}")
                    for j in range(4):
                        nc.tensor.transpose(ptr[:, j, :], fr[:, j * P:(j + 1) * P], ident)
                    nc.vector.tensor_tensor(
                        out=x1[:, 4 * n:4 * n + 4, e * P:(e + 1) * P],
                        in0=x1[:, 4 * n:4 * n + 4, e * P:(e + 1) * P],
                        in1=ptr,
                        op=ALU.add,
                    )
                for j in range(4):
                    t = 4 * n + j
                    nc.sync.dma_start(out=out_r[:, t, :], in_=x1[:, t, :])


def fold_inputs(inputs):
    """Fold LN gamma/beta and v-bias into downstream weights (exact math).

    Returns the dict of effective tensors the kernel consumes.
    """
    f = {k: np.asarray(v, dtype=np.float32) for k, v in inputs.items()}
    qkw = f["ln1_g"][:, None] * f["qk_w"]
    qkb = f["ln1_b"] @ f["qk_w"]
    vw = f["ln1_g"][:, None] * f["v_w"]
    vb = f["ln1_b"] @ f["v_w"]
    # softmax rows sum to 1 => o = attn @ (v + 1 vb^T) = attn@v + vb
    pb = f["proj_b"] + vb @ f["proj_w"]
    f1w = f["ln2_g"][:, None] * f["fc1_w"]
    f1b = f["fc1_b"] + f["ln2_b"] @ f["fc1_w"]
    import ml_dtypes

    bf16 = ml_dtypes.bfloat16
    return {
        "qkw": np.ascontiguousarray(qkw.astype(bf16)),
        "qkb": np.ascontiguousarray(qkb),
        "vw": np.ascontiguousarray(vw.astype(bf16)),
        "pw": np.ascontiguousarray(f["proj_w"].astype(bf16)),
        "pb": np.ascontiguousarray(pb),
        "f1w": np.ascontiguousarray(f1w.astype(bf16)),
        "f1b": np.ascontiguousarray(f1b),
        "f2w": np.ascontiguousarray(f["fc2_w"].astype(bf16)),
        "f2b": np.ascontiguousarray(f["fc2_b"]),
    }


_INPUT_SHAPES = {
    "x": (SEQ, EMB),
    "qkw": (EMB, 2 * EMB),
    "qkb": (2 * EMB,),
    "vw": (EMB, EMB),
    "pw": (EMB, EMB),
    "pb": (EMB,),
    "f1w": (EMB, MLPD),
    "f1b": (MLPD,),
    "f2w": (MLPD, EMB),
    "f2b": (EMB,),
}

_N_CORES = 8
_compiled = {}


def _build_nc(num_devices=_N_CORES):
    import concourse.tile as tile
    from concourse import bacc

    nc = bacc.Bacc(
        "TRN2", target_bir_lowering=False, debug=False, num_devices=num_devices
    )
    _BF16_INPUTS = {"qkw", "vw", "pw", "f1w", "f2w"}
    ins = {
        name: nc.dram_tensor(
            name, list(shape), BF16 if name in _BF16_INPUTS else F32,
            kind="ExternalInput",
        ).ap()
        for name, shape in _INPUT_SHAPES.items()
    }
    out = nc.dram_tensor("out", [SEQ, EMB], F32, kind="ExternalOutput").ap()
    with tile.TileContext(nc) as tc:
        build_block(tc, {"out": out}, ins)
    nc.compile()
    return nc


def kernel(**inputs):
    """Full-input entry point: x [8, 1024, 768] + weights -> [8, 1024, 768]."""
    from concourse.bass_utils import run_bass_kernel_spmd

    if "nc" not in _compiled:
        _compiled["nc"] = _build_nc()
    nc = _compiled["nc"]

    x = np.asarray(inputs["x"], dtype=np.float32)
    folded = fold_inputs({k: v for k, v in inputs.items() if k != "x"})
    in_maps = [
        {"x": np.ascontiguousarray(x[c]), **folded} for c in range(_N_CORES)
    ]
    res = run_bass_kernel_spmd(nc, in_maps, core_ids=list(range(_N_CORES)))
    return np.stack([res.results[c]["out"] for c in range(_N_CORES)]).astype(
        np.float32
    )



# revision 8
# speedup vs baseline: 1.1278x; 1.1278x over previous
"""Transformer block kernel for TRN2 (Bass/Tile), one batch element per core.

Computes (per core, x [1024, 768] f32):
    h  = LN(x) (gamma/beta pre-folded into weights on host)
    qk = h @ qkw + qkb ; q = qk[:, :768], k = qk[:, 768:]  (head-major 12x64)
    v  = h @ vw                 (v bias folded into proj bias on host)
    S^T[m,n] = (k_m . q_n) / 8 ;  P = exp(S^T)   (no max subtraction; scores are small)
    oe = [v; 1]^T @ P  -> rows 0..63 = unnormalized o^T, row 64 = softmax denom
    o^T = oe[0:64] / denom
    x1 = x + o @ pw + pb
    h2 = LN2(x1) (folded)
    out = x1 + gelu(h2 @ f1w + f1b) @ f2w + f2b

Layout convention: "feature-major" tensors are [feat_on_partitions, tokens] SBUF
tiles; token-major are [tokens_on_partitions, feat]. LN / residual are
token-major; matmuls contract over partitions so projections run feature-major.
"""

import sys
from contextlib import ExitStack

if "/opt/trn_rl_repo" not in sys.path:
    sys.path.insert(0, "/opt/trn_rl_repo")

import numpy as np

import concourse.bass as bass
import concourse.mybir as mybir
from concourse.masks import make_identity

F32 = mybir.dt.float32
F32R = mybir.dt.float32r
BF16 = mybir.dt.bfloat16
F8 = mybir.dt.float8e4
DR = mybir.MatmulPerfMode.DoubleRow
AF = mybir.ActivationFunctionType
ALU = mybir.AluOpType

# fp8 weight pre-scales (powers of 2, folded back out in the PSUM->SBUF stage)
QF1 = 4096.0
QF2 = 8192.0

P = 128
EMB = 768
SEQ = 1024
NH = 12
HD = 64
MLPD = 3072
EC = EMB // P      # 6 embedding chunks
NT = SEQ // P      # 8 token tiles
NC2 = SEQ // 512   # 2 token n-chunks
HC = MLPD // P     # 24 hidden chunks
HP = NH // 2       # 6 head pairs
EPS = 1e-5
SCALE = HD ** -0.5


def r32(ap):
    """Identity; matmul operands are declared float32r at allocation."""
    return ap


def _ln_stats(nc, x_ap, mv, stats, eps_t):
    """bn stats + rstd for one [128, EMB] tile; mv = [mean, rstd]."""
    xg = x_ap.rearrange("p (g d) -> p g d", d=256)
    for g in range(3):
        nc.vector.bn_stats(out=stats[:, g, :], in_=xg[:, g, :])
    nc.vector.bn_aggr(out=mv, in_=stats)
    # rstd = 1/sqrt(var + eps); Sqrt on ACT (one table set), exact recip on DVE
    # ([128,1] is one element per lane - fast)
    nc.scalar.activation(out=mv[:, 1:2], in_=mv[:, 1:2], func=AF.Sqrt, bias=eps_t, scale=1.0)
    nc.vector.reciprocal(out=mv[:, 1:2], in_=mv[:, 1:2])


def _ln_apply(nc, x_ap, h_out, mv):
    nc.vector.tensor_scalar(
        out=h_out,
        in0=x_ap,
        scalar1=mv[:, 0:1],
        scalar2=mv[:, 1:2],
        op0=ALU.subtract,
        op1=ALU.mult,
    )


def _transpose_to_featmajor(nc, tc, pool_ps, pool_sb, src_tok, dstT, t):
    """PE-transpose token-major src_tok [128, EMB] into dstT [:, e, t*128:(t+1)*128]."""
    ident = tc._block_ident
    for group_start, group_n in ((0, 4), (4, 2)):
        ptr = pool_ps.tile([P, 4 * P], BF16, tag="tr", name=f"ptr_t{t}_{group_start}")
        for j in range(group_n):
            e = group_start + j
            nc.tensor.transpose(
                ptr[:, j * P:(j + 1) * P],
                src_tok[:, e * P:(e + 1) * P],
                ident,
            )
        nc.scalar.copy(
            out=dstT[:, group_start:group_start + group_n, t * P:(t + 1) * P],
            in_=ptr[:, :group_n * P].rearrange("p (j q) -> p j q", q=P),
        )


def build_block(tc, outs, ins):
    nc = tc.nc
    x_d = ins["x"]
    qkw_d, qkb_d = ins["qkw"], ins["qkb"]
    vw_d = ins["vw"]
    pw_d, pb_d = ins["pw"], ins["pb"]
    f1w_d, f1b_d = ins["f1w"], ins["f1b"]
    f2w_d, f2b_d = ins["f2w"], ins["f2b"]
    out_d = outs["out"]

    with ExitStack() as ctx:
        consts = ctx.enter_context(tc.tile_pool(name="consts", bufs=1))
        ident = consts.tile([P, P], BF16)
        make_identity(nc, ident)
        tc._block_ident = ident
        eps_t = consts.tile([P, 1], F32)
        nc.vector.memset(eps_t, EPS)
        qkb_sb = consts.tile([P, 2 * EC], F32)
        pb_sb = consts.tile([P, EC], F32)
        f1b_sb = consts.tile([P, HC], F32)
        f2b_sb = consts.tile([P, EC], F32)

        # Persistent SBUF tensors
        glob = ctx.enter_context(tc.tile_pool(name="glob", bufs=1))
        x1 = glob.tile([P, NT, EMB], F32)            # residual stream (starts as x)
        actT = glob.tile([P, EC, SEQ], BF16, tag="actT")  # hT (LN1, attention input)
        h2T = glob.tile([P, EC, SEQ], F8, tag="h2T")  # LN2 out, fp8 for DR MLP

        attn_glob = ctx.enter_context(tc.tile_pool(name="attn_glob", bufs=1))
        vext = attn_glob.tile([P, NT, NH, HD + 1], BF16)
        oT = attn_glob.tile([P, EC, SEQ], BF16)      # attention out, feature-major
        vw_sb = attn_glob.tile([P, EC, EMB], BF16)
        pw_sb = attn_glob.tile([P, EC, EMB], BF16)

        work = ctx.enter_context(tc.tile_pool(name="work", bufs=3))
        stat_pool = ctx.enter_context(tc.tile_pool(name="stat", bufs=4))

        # ---- load x into x1 (x1 is BOTH the LN1 input and the residual acc) ----
        x_r = x_d.rearrange("(t p) e -> p t e", p=P)
        for t in range(NT):
            nc.sync.dma_start(out=x1[:, t, :], in_=x_r[:, t, :])

        # ================= Phase A: LN1 + transpose to hT =================
        with tc.tile_pool(name="psA", space="PSUM", bufs=2) as psA:
            hs, mvs = [], []
            for t in range(NT):
                mv = stat_pool.tile([P, 2], F32, tag="mv", bufs=NT, name=f"mv1_{t}")
                stats = stat_pool.tile([P, 3, 6], F32, tag="stats", name=f"st1_{t}")
                _ln_stats(nc, x1[:, t, :], mv, stats, eps_t)
                mvs.append(mv)
            for t in range(NT):
                h_t = work.tile([P, EMB], BF16, tag="h", bufs=NT, name=f"h_{t}")
                _ln_apply(nc, x1[:, t, :], h_t, mvs[t])
                hs.append(h_t)
            for t in range(NT):
                _transpose_to_featmajor(nc, tc, psA, work, hs[t], actT, t)

        # weights / biases (emitted after x+LN so the x DMAs win the queues)
        nc.sync.dma_start(out=vw_sb, in_=vw_d.rearrange("(kc p) o -> p kc o", p=P))
        nc.sync.dma_start(out=qkb_sb, in_=qkb_d.rearrange("(m p) -> p m", p=P))
        nc.sync.dma_start(out=pb_sb, in_=pb_d.rearrange("(m p) -> p m", p=P))
        nc.sync.dma_start(out=f1b_sb, in_=f1b_d.rearrange("(m p) -> p m", p=P))
        nc.sync.dma_start(out=f2b_sb, in_=f2b_d.rearrange("(m p) -> p m", p=P))

        # ================= Phase B: v projection + attention =================
        with tc.tile_pool(name="psB", space="PSUM", bufs=2) as psB:
            # ---- v = h @ vw (token-major), packed into vext with ones column ----
            nc.vector.memset(vext[:, :, :, HD:HD + 1], 1.0)
            for t in range(NT):
                pv = psB.tile([P, 2, 512], F32, tag="mm2", name=f"pv_{t}")
                for half, (c0, cw) in enumerate(((0, 512), (512, 256))):
                    for e in range(EC):
                        nc.tensor.matmul(
                            pv[:, half, :cw],
                            actT[:, e, t * P:(t + 1) * P],
                            vw_sb[:, e, c0:c0 + cw],
                            start=(e == 0),
                            stop=(e == EC - 1),
                        )
                nc.vector.tensor_copy(
                    out=vext[:, t, 0:8, 0:HD],
                    in_=pv[:, 0, :].rearrange("p (h d) -> p h d", d=HD),
                )
                nc.vector.tensor_copy(
                    out=vext[:, t, 8:12, 0:HD],
                    in_=pv[:, 1, 0:256].rearrange("p (h d) -> p h d", d=HD),
                )

            nc.sync.dma_start(out=pw_sb, in_=pw_d.rearrange("(kc p) e -> p kc e", p=P))

            # ---- per head-pair: qk projection (prefetched one pair ahead),
            # ---- then attention for 2 heads
            qkw_r = qkw_d.rearrange("(kc p) o -> p kc o", p=P)

            def emit_qk(hp):
                qkT = {}
                for role, m in (("q", hp), ("k", HP + hp)):
                    wch = work.tile([P, EC, P], BF16, tag="wchunk", name=f"qkw_{role}{hp}")
                    nc.sync.dma_start(out=wch, in_=qkw_r[:, :, m * P:(m + 1) * P])
                    dst = work.tile([P, SEQ], BF16, tag="qkT", bufs=4, name=f"{role}T_{hp}")
                    for n in range(NC2):
                        pqk = psB.tile([P, 512], F32, tag="pqk", bufs=2, name=f"pqk_{role}{hp}n{n}")
                        for e in range(EC):
                            nc.tensor.matmul(
                                pqk,
                                wch[:, e, :],
                                actT[:, e, n * 512:(n + 1) * 512],
                                start=(e == 0),
                                stop=(e == EC - 1),
                            )
                        nc.vector.tensor_scalar_add(
                            out=dst[:, n * 512:(n + 1) * 512],
                            in0=pqk,
                            scalar1=qkb_sb[:, m:m + 1],
                        )
                    qkT[role] = dst
                return qkT

            qkT = emit_qk(0)
            for hp in range(HP):
                cur = qkT
                if hp + 1 < HP:
                    qkT = emit_qk(hp + 1)

                ous = {}
                dpack = stat_pool.tile([4, 512], F32, tag="dpack", bufs=2, name=f"dp_{hp}")
                for sub in range(2):
                    h = 2 * hp + sub
                    doff = sub * HD
                    qs = cur["q"][doff:doff + HD, :]
                    ks = cur["k"][doff:doff + HD, :]
                    po = [
                        psB.tile([P, 512], F32, tag="oacc", bufs=2, name=f"po_h{h}n{n}")
                        for n in range(NC2)
                    ]
                    for mt in range(NT):
                        ps = psB.tile([P, 2, 512], F32, tag="mm2", name=f"ps_h{h}m{mt}")
                        for n in range(NC2):
                            nc.tensor.matmul(
                                ps[:, n, :],
                                ks[:, mt * P:(mt + 1) * P],
                                qs[:, n * 512:(n + 1) * 512],
                                start=True,
                                stop=True,
                            )
                        pp = work.tile([P, 2, 512], BF16, tag="ppair", bufs=4, name=f"pp_h{h}m{mt}")
                        nc.scalar.activation(out=pp, in_=ps, func=AF.Exp, scale=SCALE)
                        for n in range(NC2):
                            nc.tensor.matmul(
                                po[n][0:HD + 1, :],
                                vext[:, mt, h, :],
                                pp[:, n, :],
                                start=(mt == 0),
                                stop=(mt == NT - 1),
                            )
                    for n in range(NC2):
                        # copy out of PSUM right away so the accumulator bank
                        # recycles without waiting on the normalize chain
                        ou = work.tile([HD + 1, 512], F32, tag="ou", bufs=8, name=f"ou_h{h}n{n}")
                        nc.vector.tensor_copy(out=ou, in_=po[n][0:HD + 1, :])
                        idx = sub * NC2 + n
                        nc.sync.dma_start(out=dpack[idx:idx + 1, :], in_=ou[HD:HD + 1, :])
                        ous[idx] = ou
                # one exact reciprocal for the whole head-pair's denominators
                rpack = stat_pool.tile([4, 512], F32, tag="rpack", bufs=2, name=f"rp_{hp}")
                nc.vector.reciprocal(out=rpack, in_=dpack)
                for sub in range(2):
                    doff = sub * HD
                    for n in range(NC2):
                        idx = sub * NC2 + n
                        rtmp = stat_pool.tile([1, 512], F32, tag="rtmp", bufs=4, name=f"rt_{hp}i{idx}")
                        nc.sync.dma_start(out=rtmp, in_=rpack[idx:idx + 1, :])
                        rb = work.tile([HD, 512], F32, tag="rb", bufs=4, name=f"rb_{hp}i{idx}")
                        nc.gpsimd.partition_broadcast(rb, rtmp)
                        nc.vector.tensor_tensor(
                            out=oT[doff:doff + HD, hp, n * 512:(n + 1) * 512],
                            in0=ous[idx][0:HD, :],
                            in1=rb,
                            op=ALU.mult,
                        )

        # ====== Phase C: proj + residual + LN2, one 512-token chunk at a time ======
        with tc.tile_pool(name="psC", space="PSUM", bufs=2) as psC:
            for n in range(NC2):
                for me in range(EC):
                    ppr = psC.tile([P, 512], F32, tag="mm", name=f"ppr_{me}_{n}")
                    for kc in range(EC):
                        nc.tensor.matmul(
                            ppr,
                            pw_sb[:, kc, me * P:(me + 1) * P],
                            oT[:, kc, n * 512:(n + 1) * 512],
                            start=(kc == 0),
                            stop=(kc == EC - 1),
                        )
                    prn = work.tile([P, 512], BF16, tag="prn", name=f"prn_{me}_{n}")
                    nc.scalar.activation(
                        out=prn, in_=ppr, func=AF.Identity, bias=pb_sb[:, me:me + 1]
                    )
                    ptr = psC.tile([P, 4, P], BF16, tag="tr", name=f"trp_{me}_{n}")
                    for j in range(4):
                        nc.tensor.transpose(ptr[:, j, :], prn[:, j * P:(j + 1) * P], ident)
                    nc.vector.tensor_tensor(
                        out=x1[:, 4 * n:4 * n + 4, me * P:(me + 1) * P],
                        in0=x1[:, 4 * n:4 * n + 4, me * P:(me + 1) * P],
                        in1=ptr,
                        op=ALU.add,
                    )
                hs2, mvs2 = [], []
                for j in range(4):
                    t = 4 * n + j
                    mv = stat_pool.tile([P, 2], F32, tag="mv", bufs=NT, name=f"mv2_{t}")
                    stats = stat_pool.tile([P, 3, 6], F32, tag="stats", name=f"st2_{t}")
                    _ln_stats(nc, x1[:, t, :], mv, stats, eps_t)
                    mvs2.append(mv)
                for j in range(4):
                    t = 4 * n + j
                    h_t = work.tile([P, EMB], BF16, tag="h", bufs=NT, name=f"h2_{t}")
                    _ln_apply(nc, x1[:, t, :], h_t, mvs2[j])
                    hs2.append(h_t)
                for j in range(4):
                    _transpose_to_featmajor(nc, tc, psC, work, hs2[j], h2T, 4 * n + j)

        # ================= Phase F: MLP + residual + output =================
        # fp8 DoubleRow: K=256 per matmul (kc pairs for fc1, hc pairs for fc2)
        out_r = out_d.rearrange("(t p) e -> p t e", p=P)
        f1w_r = f1w_d.rearrange("(kc p) o -> p kc o", p=P)
        f2w_r = f2w_d.rearrange("(hc p) e -> p hc e", p=P)
        HCP = HC // 2  # 12 hc pairs
        with tc.tile_pool(name="psF", space="PSUM", bufs=1) as psF:
            for n in range(NC2):
                acc = [
                    psF.tile([P, 2, 512], F32, tag=f"acc{i}", bufs=1, name=f"acc_{n}_{i}")
                    for i in range(3)
                ]

                def acc_sl(e):
                    return acc[e // 2][:, e % 2, :]

                for hcp in range(HCP):
                    w2 = work.tile([P, 2, EMB], F8, tag="w2chunk", name=f"f2w_{n}_{hcp}")
                    nc.sync.dma_start(out=w2, in_=f2w_r[:, 2 * hcp:2 * hcp + 2, :])
                    a2 = work.tile([P, 2, 512], F8, tag="act", name=f"act_{n}_{hcp}")
                    for sub in range(2):
                        hc = 2 * hcp + sub
                        w1 = work.tile([P, EC, P], F8, tag="wchunk", name=f"f1w_{n}_{hc}")
                        nc.sync.dma_start(out=w1, in_=f1w_r[:, :, hc * P:(hc + 1) * P])
                        pf1 = psF.tile([P, 512], F32, tag="f1", bufs=2, name=f"pf1_{n}_{hc}")
                        for i in range(EC // 2):
                            nc.tensor.matmul(
                                pf1,
                                w1[:, 2 * i:2 * i + 2, :],
                                h2T[:, 2 * i:2 * i + 2, n * 512:(n + 1) * 512],
                                start=(i == 0),
                                stop=(i == EC // 2 - 1),
                                perf_mode=DR,
                            )
                        nc.scalar.activation(
                            out=a2[:, sub, :], in_=pf1, func=AF.Gelu,
                            bias=f1b_sb[:, hc:hc + 1], scale=1.0 / QF1,
                        )
                    for e in range(EC):
                        nc.tensor.matmul(
                            acc_sl(e),
                            w2[:, :, e * P:(e + 1) * P],
                            a2,
                            start=(hcp == 0),
                            stop=(hcp == HCP - 1),
                            perf_mode=DR,
                        )
                for e in range(EC):
                    fr = work.tile([P, 512], BF16, tag="prn", name=f"fr_{n}_{e}")
                    nc.scalar.activation(
                        out=fr, in_=acc_sl(e), func=AF.Identity,
                        bias=f2b_sb[:, e:e + 1], scale=1.0 / QF2,
                    )
                    ptr = psF.tile([P, 4, P], BF16, tag="f1", bufs=2, name=f"trf_{n}_{e}")
                    for j in range(4):
                        nc.tensor.transpose(ptr[:, j, :], fr[:, j * P:(j + 1) * P], ident)
                    nc.vector.tensor_tensor(
                        out=x1[:, 4 * n:4 * n + 4, e * P:(e + 1) * P],
                        in0=x1[:, 4 * n:4 * n + 4, e * P:(e + 1) * P],
                        in1=ptr,
                        op=ALU.add,
                    )
                for j in range(4):
                    t = 4 * n + j
                    nc.sync.dma_start(out=out_r[:, t, :], in_=x1[:, t, :])


def fold_inputs(inputs):
    """Fold LN gamma/beta and v-bias into downstream weights (exact math).

    Returns the dict of effective tensors the kernel consumes.
    """
    f = {k: np.asarray(v, dtype=np.float32) for k, v in inputs.items()}
    qkw = f["ln1_g"][:, None] * f["qk_w"]
    qkb = f["ln1_b"] @ f["qk_w"]
    vw = f["ln1_g"][:, None] * f["v_w"]
    vb = f["ln1_b"] @ f["v_w"]
    # softmax rows sum to 1 => o = attn @ (v + 1 vb^T) = attn@v + vb
    pb = f["proj_b"] + vb @ f["proj_w"]
    f1w = f["ln2_g"][:, None] * f["fc1_w"]
    f1b = f["fc1_b"] + f["ln2_b"] @ f["fc1_w"]
    import ml_dtypes

    bf16 = ml_dtypes.bfloat16
    fp8 = ml_dtypes.float8_e4m3fn

    def q8(w, s):
        return np.ascontiguousarray(np.clip(w * s, -240.0, 240.0).astype(fp8))

    return {
        "qkw": np.ascontiguousarray(qkw.astype(bf16)),
        "qkb": np.ascontiguousarray(qkb),
        "vw": np.ascontiguousarray(vw.astype(bf16)),
        "pw": np.ascontiguousarray(f["proj_w"].astype(bf16)),
        "pb": np.ascontiguousarray(pb),
        "f1w": q8(f1w, QF1),
        "f1b": np.ascontiguousarray(f1b),
        "f2w": q8(f["fc2_w"], QF2),
        "f2b": np.ascontiguousarray(f["fc2_b"]),
    }


_INPUT_SHAPES = {
    "x": (SEQ, EMB),
    "qkw": (EMB, 2 * EMB),
    "qkb": (2 * EMB,),
    "vw": (EMB, EMB),
    "pw": (EMB, EMB),
    "pb": (EMB,),
    "f1w": (EMB, MLPD),
    "f1b": (MLPD,),
    "f2w": (MLPD, EMB),
    "f2b": (EMB,),
}

_N_CORES = 8
_compiled = {}


def _build_nc(num_devices=_N_CORES):
    import concourse.tile as tile
    from concourse import bacc

    nc = bacc.Bacc(
        "TRN2", target_bir_lowering=False, debug=False, num_devices=num_devices
    )
    _BF16_INPUTS = {"qkw", "vw", "pw"}
    _FP8_INPUTS = {"f1w", "f2w"}
    ins = {
        name: nc.dram_tensor(
            name, list(shape),
            BF16 if name in _BF16_INPUTS else (F8 if name in _FP8_INPUTS else F32),
            kind="ExternalInput",
        ).ap()
        for name, shape in _INPUT_SHAPES.items()
    }
    out = nc.dram_tensor("out", [SEQ, EMB], F32, kind="ExternalOutput").ap()
    with tile.TileContext(nc) as tc:
        build_block(tc, {"out": out}, ins)
    nc.compile()
    return nc


def kernel(**inputs):
    """Full-input entry point: x [8, 1024, 768] + weights -> [8, 1024, 768]."""
    from concourse.bass_utils import run_bass_kernel_spmd

    if "nc" not in _compiled:
        _compiled["nc"] = _build_nc()
    nc = _compiled["nc"]

    x = np.asarray(inputs["x"], dtype=np.float32)
    folded = fold_inputs({k: v for k, v in inputs.items() if k != "x"})
    in_maps = [
        {"x": np.ascontiguousarray(x[c]), **folded} for c in range(_N_CORES)
    ]
    res = run_bass_kernel_spmd(nc, in_maps, core_ids=list(range(_N_CORES)))
    return np.stack([res.results[c]["out"] for c in range(_N_CORES)]).astype(
        np.float32
    )

